# revision 1
# baseline (speedup 1.0000x reference)
"""Trainium2 Bass kernel for nn_AMIML_5102421148136 (topk_masking).

Data-parallel over batch B=8 across 8 NeuronCores. Each core runs, for its
sample:
  Phase A (bulk): MLP 256->128->64->32 (BN folded, relu) -> score projection
    (conv4+conv5 folded) over all 100k positions, bf16 compute with fp32
    accumulation. Scores written to a DRAM scratch.
  Phase B: gpsimd global top-256 of s and of -s; gather the 512 candidate
    x-rows; recompute the MLP on candidates in fp32(r); exact top/bottom-100
    selection among candidates (max8/match_replace rounds); tiny 200-token
    attention; final softmax -> out row [1, 8].
Host stacks the 8 per-core rows into the [8, 8] output.
"""

import sys
import math

if '/opt/trn_rl_repo' not in sys.path:
    sys.path.insert(0, '/opt/trn_rl_repo')

import numpy as np
import ml_dtypes

import concourse.bass as bass
import concourse.mybir as mybir
from concourse.tile import TileContext
from concourse.tile_rust import add_dep_helper
from concourse import bass_utils, bacc

F32 = mybir.dt.float32
F32R = mybir.dt.float32r
BF16 = mybir.dt.bfloat16
U32 = mybir.dt.uint32
AF = mybir.ActivationFunctionType
ALU = mybir.AluOpType

# ---- problem constants (hardcoded; kernel.py must be self-contained) ----
B = 8
N = 100000
NPAD = 100352          # 16 * 6272 = 112 * 896, multiple of 128
C = 256
TILE = 896             # positions per phase-A tile (7 * 128)
NT = NPAD // TILE      # 112
GRP = 7                # tiles per topk partition chunk (7*896 = 6272)
CHUNK = NPAD // 16     # 6272 positions per topk partition
D = 8
R = 100
ALPHA = 0.1
EPS = 1e-5
NEG = -1.0e30
TOK = 256              # padded token count for attention (200 real)
HALF = NPAD // 2       # 50176, topk vocab per token (ISA limit: u16)
CHUNK2 = HALF // 16    # 3136
NCAND = 512            # 4 tokens x top-128 kept

_CACHE = {}
LAST = {}


def _fold_weights(inputs):
    """Host-side exact algebra: BN fold, conv4+conv5 fold, bias augmentation."""
    w = {}

    def fold(li):
        cw = inputs[f'conv{li}_w'].astype(np.float64)
        cb = inputs[f'conv{li}_b'].astype(np.float64)
        g = inputs[f'bn{li}_g'].astype(np.float64)
        bb = inputs[f'bn{li}_b'].astype(np.float64)
        m = inputs[f'bn{li}_m'].astype(np.float64)
        v = inputs[f'bn{li}_v'].astype(np.float64)
        sc = g / np.sqrt(v + EPS)
        Wf = (cw * sc[:, None]).T          # [cin, cout]
        bf = (cb - m) * sc + bb
        return Wf.astype(np.float32), bf.astype(np.float32)

    W1, b1 = fold(1)
    W2, b2 = fold(2)
    W3, b3 = fold(3)
    W4 = inputs['conv4_w'].T.astype(np.float32)       # [32, 8]
    b4 = inputs['conv4_b'].astype(np.float32)         # [8]
    w5 = inputs['conv5_w'][0].astype(np.float32)      # [8]
    b5 = float(inputs['conv5_b'][0])
    Ws = (W4.astype(np.float64) @ w5.astype(np.float64)).astype(np.float32)  # [32]
    bs = float(b4.astype(np.float64) @ w5.astype(np.float64) + b5)

    # phase A (bf16) weights, SBUF layouts
    w['w1'] = W1.reshape(2, 128, 128).transpose(1, 0, 2).astype(ml_dtypes.bfloat16)  # [128,2,128]
    w['w2'] = W2.astype(ml_dtypes.bfloat16)            # [128, 64]
    w['w3'] = W3.astype(ml_dtypes.bfloat16)            # [64, 32]
    w['wsc'] = Ws.reshape(32, 1).astype(ml_dtypes.bfloat16)  # [32, 1]
    w['b1'] = b1.reshape(128, 1)
    w['b2'] = b2.reshape(64, 1)
    w['b3'] = b3.reshape(32, 1)

    # refine (fp32) weights
    w['rw1'] = W1.reshape(2, 128, 128).transpose(1, 0, 2).copy()  # [128,2,128] f32
    w['rw2'] = W2.copy()                               # [128, 64]
    w['rw3'] = W3.copy()                               # [64, 32]
    rw45 = np.concatenate([W4, Ws.reshape(32, 1)], axis=1)  # [32, 9]
    w['rw45'] = rw45.astype(np.float32)
    rb45 = np.concatenate([b4, np.array([bs], np.float32)]).reshape(9, 1)
    w['rb45'] = rb45.astype(np.float32)

    # attention weights. scores scaled by 1/sqrt(D) folded into q.
    sq = 1.0 / math.sqrt(D)
    qw = inputs['q_w'].astype(np.float32) * sq         # [8, 8] (out, in)
    qb = inputs['q_b'].astype(np.float32) * sq
    kw = inputs['k_w'].astype(np.float32)
    kb = inputs['k_b'].astype(np.float32)
    vw = inputs['v_w'].astype(np.float32)
    vb = inputs['v_b'].astype(np.float32)
    # augmented lhsT [9, 8]: rows 0..7 = W.T (d, e), row 8 = bias
    w['qw'] = np.concatenate([qw.T, qb.reshape(1, 8)], axis=0).astype(np.float32)
    w['kw'] = np.concatenate([kw.T, kb.reshape(1, 8)], axis=0).astype(np.float32)
    w['vw'] = np.concatenate([vw.T, vb.reshape(1, 8)], axis=0).astype(np.float32)
    w['w6'] = (ALPHA * inputs['conv6_w'].astype(np.float32)).reshape(1, 8)
    w['b6'] = (ALPHA * inputs['conv6_b'].astype(np.float32)).reshape(1, 8)

    # constants
    ident = np.eye(128, dtype=np.float32)
    w['ident'] = ident
    w['ident16'] = ident.astype(ml_dtypes.bfloat16)
    onesmask = np.zeros((128, 2), np.float32)
    onesmask[:, 0] = 1.0
    onesmask[:72, 1] = 1.0
    w['onesmask'] = onesmask
    w['c256'] = np.array([[0.0], [256.0]], np.float32)  # per-partition col offset
    w['negrow'] = np.full((1, 352), NEG, np.float32)
    offc = np.zeros((128, 4), np.float32)
    for tcol in range(4):
        offc[:, tcol] = float(tcol % 2) * (100352 // 2)
    w['offc'] = offc
    w['sgn'] = np.array([[1.0], [-1.0]], np.float32)   # val sign per side
    return w


WEIGHT_SPECS = [
    ('w1', [128, 2, 128], BF16), ('w2', [128, 64], BF16), ('w3', [64, 32], BF16),
    ('wsc', [32, 1], BF16),
    ('b1', [128, 1], F32), ('b2', [64, 1], F32), ('b3', [32, 1], F32),
    ('rw1', [128, 2, 128], F32), ('rw2', [128, 64], F32), ('rw3', [64, 32], F32),
    ('rw45', [32, 9], F32), ('rb45', [9, 1], F32),
    ('qw', [9, 8], F32), ('kw', [9, 8], F32), ('vw', [9, 8], F32),
    ('w6', [1, 8], F32), ('b6', [1, 8], F32),
    ('ident', [128, 128], F32), ('ident16', [128, 128], BF16),
    ('onesmask', [128, 2], F32),
    ('c256', [2, 1], F32), ('sgn', [2, 1], F32), ('offc', [128, 4], F32),
    ('negrow', [1, 352], F32),
]


def build_bass():
    nc = bacc.Bacc("TRN2", target_bir_lowering=False, debug=False)

    x_d = nc.dram_tensor("x", [N, C], F32, kind="ExternalInput")
    wd = {}
    for name, shape, dt in WEIGHT_SPECS:
        wd[name] = nc.dram_tensor(name, shape, dt, kind="ExternalInput")
    out_d = nc.dram_tensor("out", [1, D], F32, kind="ExternalOutput")

    s_d = nc.dram_tensor("s_scratch", [NPAD], F32, kind="Internal")
    cand_d = nc.dram_tensor("cand_scratch", [NCAND, 16], F32, kind="Internal")
    idx_d = nc.dram_tensor("idx_scratch", [1024], U32, kind="Internal")
    sc_d = nc.dram_tensor("sc_scratch", [NCAND], F32, kind="Internal")
    vi_d = nc.dram_tensor("vi_scratch", [208], U32, kind="Internal")
    vv_d = nc.dram_tensor("vv_scratch", [208], F32, kind="Internal")
    wb_d = nc.dram_tensor("w_scratch", [256], F32, kind="Internal")

    with TileContext(nc) as tc:
        with tc.tile_pool(name="consts", bufs=1) as cpool:
            ws = {}
            for name, shape, dt in WEIGHT_SPECS:
                t = cpool.tile(shape, dt, tag=name)
                sl = t[0:shape[0]]
                nc.sync.dma_start(sl, wd[name].ap())
                ws[name] = t[0:shape[0]]

            # ---------------- Phase A ----------------
            # raw SBUF tensors (gpsimd.topk requires SBTensorHandle)
            s2_t = nc.alloc_sbuf_tensor("s2_topk", [64, CHUNK2], F32)
            s2_sb = s2_t.ap()
            tk_t = nc.alloc_sbuf_tensor("tk_out", [64, 32], U32)

            with (
                tc.tile_pool(name="xin", bufs=4) as xpool,
                tc.tile_pool(name="work", bufs=3) as wpool,
                tc.tile_pool(name="sstage", bufs=2) as spool,
                tc.tile_pool(name="ps_xt", bufs=2, space="PSUM") as ps_xt,
                tc.tile_pool(name="ps_big", bufs=1, space="PSUM") as ps_big,
                tc.tile_pool(name="ps_s", bufs=1, space="PSUM") as ps_sp,
            ):
                s_stage = None
                ps_s = ps_sp.tile([128, 1024], F32, tag="pss")
                saved = {}
                s_writes = []

                def load_x(t_i):
                    p0 = t_i * TILE
                    x_sb = xpool.tile([128, 7, C], BF16, tag="x")
                    if p0 + TILE <= N:
                        nc.gpsimd.dma_start(
                            x_sb[:],
                            x_d.ap()[p0:p0 + TILE, :].rearrange("(g p) c -> p g c", p=128),
                        )
                    else:
                        nval = N - p0          # 544 = 4*128 + 32
                        gfull = nval // 128    # 4
                        rem = nval - gfull * 128
                        nc.vector.memset(x_sb[:], 0.0)
                        nc.gpsimd.dma_start(
                            x_sb[:, :gfull],
                            x_d.ap()[p0:p0 + gfull * 128, :].rearrange(
                                "(g p) c -> p g c", p=128),
                        )
                        if rem:
                            nc.gpsimd.dma_start(
                                x_sb[:rem, gfull],
                                x_d.ap()[p0 + gfull * 128:p0 + nval, :],
                            )
                    return x_sb

                for t_i in range(NT + 1):
                    cur = t_i < NT
                    prv = t_i >= 1

                    if prv:
                        # L2 (t_i-1): [128 -> 64]
                        h1p = saved['h1']
                        ph2 = ps_big.tile([64, TILE], F32, tag="ps23")
                        for nn_ in range(2):
                            sl = slice(nn_ * 448, (nn_ + 1) * 448)
                            nc.tensor.matmul(ph2[:, sl], lhsT=ws['w2'],
                                             rhs=h1p[:, sl], start=True, stop=True)
                        h2 = wpool.tile([64, TILE], BF16, tag="h2")
                        nc.scalar.activation(h2[:], ph2[:], AF.Relu, bias=ws['b2'])

                    if cur:
                        x_sb = load_x(t_i)
                        # transpose x tile to channel-major bf16 via PE
                        xT = wpool.tile([128, 2, TILE], BF16, tag="xT")
                        for cc in range(2):
                            pst = ps_xt.tile([128, TILE], BF16, tag="psxT")
                            for g in range(7):
                                nc.tensor.transpose(
                                    pst[:, g * 128:(g + 1) * 128],
                                    x_sb[:, g, cc * 128:(cc + 1) * 128],
                                    ws['ident16'],
                                )
                            nc.vector.tensor_copy(xT[:, cc], pst[:])

                    if prv:
                        # L3 (t_i-1): [64 -> 32]
                        ph3 = ps_big.tile([32, TILE], F32, tag="ps23")
                        for nn_ in range(2):
                            sl = slice(nn_ * 448, (nn_ + 1) * 448)
                            nc.tensor.matmul(ph3[:, sl], lhsT=ws['w3'],
                                             rhs=h2[:, sl], start=True, stop=True)
                        h3 = wpool.tile([32, TILE], BF16, tag="h3")
                        nc.vector.tensor_scalar(h3[:], ph3[:], ws['b3'], 0.0,
                                                op0=ALU.add, op1=ALU.max)

                    if cur:
                        # L1 (t_i): [256 -> 128]
                        ph1 = ps_big.tile([128, TILE], F32, tag="ps1")
                        for cc in range(2):
                            for nn_ in range(2):
                                sl = slice(nn_ * 448, (nn_ + 1) * 448)
                                nc.tensor.matmul(
                                    ph1[:, sl], lhsT=ws['w1'][:, cc],
                                    rhs=xT[:, cc, sl],
                                    start=(cc == 0), stop=(cc == 1),
                                )
                        h1 = wpool.tile([128, TILE], BF16, tag="h1")
                        nc.scalar.activation(h1[:], ph1[:], AF.Relu, bias=ws['b1'])
                        saved['h1'] = h1

                    if prv:
                        tp = t_i - 1
                        # score projection [32 -> 1]: chunk c of the current
                        # 4-tile group at psum partition 32*(c%4), bank c//4
                        if tp == 0:
                            nc.vector.memset(ps_s[:], 0.0)
                        for nn_ in range(2):
                            sl = slice(nn_ * 448, (nn_ + 1) * 448)
                            c = 2 * (tp % 4) + nn_
                            pb_ = 32 * (c % 4)
                            bk = c // 4
                            nc.tensor.matmul(
                                ps_s[pb_:pb_ + 1, bk * 512:bk * 512 + 448],
                                lhsT=ws['wsc'],
                                rhs=h3[:, sl], start=True, stop=True,
                                tile_position=(0, pb_),
                            )
                        if tp % 4 == 3:
                            gi = tp // 4
                            s_stage = spool.tile([128, 2, 448], F32, tag="sstage")
                            for bk_ in range(2):
                                nc.vector.tensor_copy(
                                    s_stage[:, bk_],
                                    ps_s[:, bk_ * 512:bk_ * 512 + 448])
                            sd_ap = s_d.ap()
                            for p4 in range(4):
                                off = gi * 3584 + p4 * 448
                                w_i = nc.sync.dma_start(
                                    bass.AP(sd_ap.tensor, off, [[1792, 2], [1, 448]]),
                                    s_stage[32 * p4:32 * p4 + 1, :, :],
                                )
                                s_writes.append(w_i.ins)

            # ---------------- Phase B ----------------
            with (
                tc.tile_pool(name="pb", bufs=1) as pb,
                tc.tile_pool(name="ps_b", bufs=1, space="PSUM") as psb,
            ):
                # topk input: tokens 0,1 = s halves; tokens 2,3 = -s halves
                ha = s_d.ap().rearrange("(p f) -> p f", p=32)
                ld0 = nc.sync.dma_start(s2_sb[0:32], ha)
                ld1 = nc.sync.dma_start(s2_sb[32:64], ha)
                for w_ in s_writes:
                    add_dep_helper(ld0.ins, w_, reason="s2 load after score writes")
                    add_dep_helper(ld1.ins, w_, reason="s2 load after score writes")
                neg_i = nc.vector.tensor_scalar_mul(s2_sb[32:64], s2_sb[32:64], -1.0)
                add_dep_helper(neg_i.ins, ld0.ins, reason="neg after load")
                add_dep_helper(neg_i.ins, ld1.ins, reason="neg after load")
                pad0 = N - 31 * CHUNK2   # 2784: valid prefix in partition 31/63
                # DVE can't address partition bases 31/63; patch pads via DMA
                pt0 = nc.sync.dma_start(s2_sb[31:32, pad0:], ws['negrow'])
                pt1 = nc.sync.dma_start(s2_sb[63:64, pad0:], ws['negrow'])
                add_dep_helper(pt0.ins, ld0.ins, reason="patch after load")
                add_dep_helper(pt1.ins, neg_i.ins, reason="patch after neg")

                tk = tk_t.ap()
                tk_i = nc.gpsimd.topk(tk, s2_sb, tokens=4, vocab_size=HALF, k=256)
                for d_ in (ld0, ld1, neg_i, pt0, pt1):
                    add_dep_helper(tk_i.ins, d_.ins, reason="topk after s2 ready")

                # rearrange candidate indices via DRAM bounce; keep each
                # token's top-128 (ascending sort: slots 128..255)
                wi_ = nc.sync.dma_start(
                    idx_d.ap().rearrange("(p f) -> p f", p=64), tk[:, 16:32])
                add_dep_helper(wi_.ins, tk_i.ins, reason="idx write after topk")
                idxg = pb.tile([128, 4], U32)
                ida = idx_d.ap()
                ri_ = nc.sync.dma_start(
                    idxg[:], bass.AP(ida.tensor, 128, [[1, 128], [256, 4]]))
                add_dep_helper(ri_.ins, wi_.ins, reason="idx bounce order")
                # add per-half position offset (via f32; values < 2^24 exact)
                idxf = pb.tile([128, 4], F32)
                nc.vector.tensor_copy(idxf[:], idxg[:])
                nc.vector.tensor_add(idxf[:], idxf[:], ws['offc'])
                nc.vector.tensor_copy(idxg[:], idxf[:])

                # gather candidate x rows (512 rows of 256 floats)
                xg = pb.tile([128, 4, C], F32)
                xg_gathers = []
                for tcol in range(4):
                    g0 = nc.gpsimd.indirect_dma_start(
                        out=xg[:, tcol], out_offset=None,
                        in_=x_d.ap(),
                        in_offset=bass.IndirectOffsetOnAxis(ap=idxg[:, tcol:tcol + 1], axis=0),
                    )
                    xg_gathers.append(g0)

                # transpose candidates to channel-major fp32
                xgT = pb.tile([128, 2, NCAND], F32)
                for ch in range(1):
                    for cc in range(2):
                        pst = psb.tile([128, 512], F32, tag="psb512")
                        for tq in range(4):
                            tcol = ch * 4 + tq
                            tr_i = nc.tensor.transpose(
                                pst[:, tq * 128:(tq + 1) * 128],
                                xg[:, tcol, cc * 128:(cc + 1) * 128],
                                ws['ident'],
                            )
                            add_dep_helper(tr_i.ins, xg_gathers[tcol].ins,
                                           reason="transpose after gather")
                        nc.vector.tensor_copy(
                            xgT[:, cc, ch * 512:(ch + 1) * 512], pst[:])

                # refine MLP in fp32, 512-wide chunks
                r45 = pb.tile([128, NCAND], F32)  # rows 0..7 x4T, row 8 exact s
                nc.vector.memset(r45[:], 0.0)
                for ch in range(1):
                    sl = slice(ch * 512, (ch + 1) * 512)
                    ps1 = psb.tile([128, 512], F32, tag="psb512")
                    for cc in range(2):
                        nc.tensor.matmul(
                            ps1[:], lhsT=ws['rw1'][:, cc],
                            rhs=xgT[:, cc, sl],
                            start=(cc == 0), stop=(cc == 1))
                    rh1 = pb.tile([128, 512], F32, tag="rh1")
                    nc.scalar.activation(rh1[:], ps1[:], AF.Relu, bias=ws['b1'])

                    ps2 = psb.tile([64, 512], F32, tag="psb512")
                    nc.tensor.matmul(ps2[:], lhsT=ws['rw2'],
                                     rhs=rh1[:], start=True, stop=True)
                    rh2 = pb.tile([64, 512], F32, tag="rh2")
                    nc.scalar.activation(rh2[:], ps2[:], AF.Relu, bias=ws['b2'])

                    ps3 = psb.tile([32, 512], F32, tag="psb512")
                    nc.tensor.matmul(ps3[:], lhsT=ws['rw3'],
                                     rhs=rh2[:], start=True, stop=True)
                    rh3 = pb.tile([32, 512], F32, tag="rh3")
                    nc.scalar.activation(rh3[:], ps3[:], AF.Relu, bias=ws['b3'])

                    ps4 = psb.tile([9, 512], F32, tag="psb512")
                    nc.tensor.matmul(ps4[:], lhsT=ws['rw45'],
                                     rhs=rh3[:], start=True, stop=True)
                    nc.vector.tensor_scalar(r45[0:9, sl], ps4[:], ws['rb45'],
                                            None, op0=ALU.add)

                # store candidates row-major for the column gather:
                # [512 cand, 16] (cols 0..8 = x4|s, rest pad)
                r45T = pb.tile([128, 4, 16], F32)
                nc.vector.memset(r45T[:], 0.0)
                for ch in range(1):
                    pst45 = psb.tile([128, 512], F32, tag="psb512")
                    for tq in range(4):
                        tcol = ch * 4 + tq
                        nc.tensor.transpose(
                            pst45[:, tq * 128:(tq + 1) * 128],
                            r45[:, tcol * 128:(tcol + 1) * 128],
                            ws['ident'],
                        )
                    for tq in range(4):
                        tcol = ch * 4 + tq
                        nc.vector.tensor_copy(
                            r45T[:, tcol, 0:9], pst45[:, tq * 128:tq * 128 + 9])
                candw = nc.sync.dma_start(
                    cand_d.ap().rearrange("(t p) f -> p t f", p=128), r45T[:])

                # exact select: top-100 of s (row 0) and of -s (row 1) among
                # 256 candidates each
                s2c = pb.tile([2, 256], F32)
                scw = nc.sync.dma_start(sc_d.ap().rearrange("(o f) -> o f", o=1), r45[8:9, :])
                scr = nc.sync.dma_start(s2c[:], sc_d.ap().rearrange("(t c) -> t c", t=2))
                add_dep_helper(scr.ins, scw.ins, reason="sc bounce order")
                nc.vector.tensor_scalar(s2c[:], s2c[:], ws['sgn'], None, op0=ALU.mult)

                work = pb.tile([2, 256], F32)
                nc.vector.tensor_copy(work[:], s2c[:])
                vals = pb.tile([2, 104], F32)
                cidx = pb.tile([2, 104], U32)
                for r_ in range(13):
                    sl = slice(r_ * 8, (r_ + 1) * 8)
                    nc.vector.max(out=vals[:, sl], in_=work[:])
                    nc.vector.max_index(out=cidx[:, sl], in_max=vals[:, sl],
                                        in_values=s2c[:])
                    nc.vector.match_replace(out=work[:], in_to_replace=vals[:, sl],
                                            in_values=work[:], imm_value=NEG)

                # global candidate row = side*256 + cidx ; vals row1 = -vals
                cidx_f = pb.tile([2, 104], F32)
                nc.vector.tensor_copy(cidx_f[:], cidx[:])
                nc.vector.tensor_scalar(cidx_f[:], cidx_f[:], ws['c256'], None, op0=ALU.add)
                ccol = pb.tile([2, 104], U32)
                nc.vector.tensor_copy(ccol[:], cidx_f[:])
                nc.vector.tensor_scalar(vals[:], vals[:], ws['sgn'], None, op0=ALU.mult)

                # build token-order index [128, 2] and values [128, 2]
                ccol_g = pb.tile([128, 2], U32)
                vals_g = pb.tile([128, 2], F32)
                for (dst, src_t, bd) in ((ccol_g, ccol, vi_d), (vals_g, vals, vv_d)):
                    bw_ = nc.sync.dma_start(bd.ap().rearrange("(p f) -> p f", p=2), src_t[:])
                    bda = bd.ap()
                    for (osl, isl) in (((slice(0, 100), 0), (0, 100)),
                                       ((slice(100, 128), 0), (104, 132)),
                                       ((slice(0, 72), 1), (132, 204)),
                                       ((slice(72, 128), 1), (148, 204))):
                        r_i = nc.sync.dma_start(
                            dst[osl[0], osl[1]:osl[1] + 1],
                            bda[isl[0]:isl[1]].rearrange("f -> f ()"))
                        add_dep_helper(r_i.ins, bw_.ins, reason="vi/vv bounce order")

                # gather selected candidate rows: [128, 2, 16]
                x4a = pb.tile([128, 2, 16], F32)
                x4a_gathers = []
                for tcol in range(2):
                    g_i = nc.gpsimd.indirect_dma_start(
                        out=x4a[:, tcol], out_offset=None,
                        in_=cand_d.ap(),
                        in_offset=bass.IndirectOffsetOnAxis(ap=ccol_g[:, tcol:tcol + 1], axis=0),
                    )
                    add_dep_helper(g_i.ins, candw.ins, reason="gather after cand write")
                    x4a_gathers.append(g_i)

                # h2 tokens (token-major): x4 + vals*w6 + b6 ; col 8 = 1.0
                w6b = pb.tile([128, 8], F32)
                nc.gpsimd.partition_broadcast(w6b[:], ws['w6'])
                b6b = pb.tile([128, 8], F32)
                nc.gpsimd.partition_broadcast(b6b[:], ws['b6'])

                h2a = pb.tile([128, 2, 9], F32)
                for tcol in range(2):
                    nc.vector.tensor_scalar(h2a[:, tcol, 0:8], w6b[:],
                                            vals_g[:, tcol:tcol + 1], None, op0=ALU.mult)
                    a_i = nc.vector.tensor_add(h2a[:, tcol, 0:8], h2a[:, tcol, 0:8],
                                               x4a[:, tcol, 0:8])
                    add_dep_helper(a_i.ins, x4a_gathers[tcol].ins,
                                   reason="h2 after x4a gather")
                    nc.vector.tensor_add(h2a[:, tcol, 0:8], h2a[:, tcol, 0:8], b6b[:])
                nc.vector.memset(h2a[:, :, 8:9], 1.0)

                # transpose h2 to channel-major [9, 256]
                h2T = pb.tile([9, TOK], F32)
                psh = psb.tile([16, 256], F32, tag="psb_h2t")
                for tcol in range(2):
                    nc.tensor.transpose(
                        psh[0:9, tcol * 128:(tcol + 1) * 128],
                        h2a[:, tcol], ws['ident'])
                nc.vector.tensor_copy(h2T[:], psh[0:9, :])

                # q, k (channel-major) and v (token-major)
                psq = psb.tile([8, TOK], F32, tag="psb_q")
                nc.tensor.matmul(psq[:], lhsT=ws['qw'], rhs=h2T[:], start=True, stop=True)
                qT = pb.tile([8, TOK], F32)
                nc.vector.tensor_copy(qT[:], psq[:])

                psk = psb.tile([8, TOK], F32, tag="psb_q")
                nc.tensor.matmul(psk[:], lhsT=ws['kw'], rhs=h2T[:], start=True, stop=True)
                kT = pb.tile([8, TOK], F32)
                nc.vector.tensor_copy(kT[:], psk[:])

                v_sb = pb.tile([128, 2, 8], F32)
                for tcol in range(2):
                    psv = psb.tile([128, 8], F32, tag="psb_v")
                    nc.tensor.matmul(psv[:], lhsT=h2T[:, tcol * 128:(tcol + 1) * 128],
                                     rhs=ws['vw'], start=True, stop=True)
                    nc.vector.tensor_copy(v_sb[:, tcol], psv[:])

                # scores + masked softmax + column sums (weights w)
                psw = psb.tile([1, TOK], F32, tag="psb_w")
                for tcol in range(2):
                    psS = psb.tile([128, TOK], F32, tag="psb_S")
                    nc.tensor.matmul(psS[:], lhsT=qT[:, tcol * 128:(tcol + 1) * 128],
                                     rhs=kT[:], start=True, stop=True)
                    nc.vector.memset(psS[:, 200:], NEG)
                    mrow = pb.tile([128, 1], F32, tag="mrow")
                    nc.vector.reduce_max(mrow[:], psS[:], axis=mybir.AxisListType.X)
                    mneg = pb.tile([128, 1], F32, tag="mneg")
                    nc.vector.tensor_scalar_mul(mneg[:], mrow[:], -1.0)
                    pexp = pb.tile([128, TOK], F32, tag="pexp")
                    sume = pb.tile([128, 1], F32, tag="sume")
                    nc.scalar.activation(pexp[:], psS[:], AF.Exp, bias=mneg[:],
                                         accum_out=sume[:])
                    rsum = pb.tile([128, 1], F32, tag="rsum")
                    nc.vector.reciprocal(rsum[:], sume[:])
                    nc.vector.tensor_mul(rsum[:], rsum[:],
                                         ws['onesmask'][:, tcol:tcol + 1])
                    nc.tensor.matmul(psw[:], lhsT=rsum[:], rhs=pexp[:],
                                     start=(tcol == 0), stop=(tcol == 1))

                w_sb = pb.tile([1, TOK], F32)
                nc.vector.tensor_copy(w_sb[:], psw[:])
                wbw = nc.sync.dma_start(wb_d.ap().rearrange("(o f) -> o f", o=1), w_sb[:])
                wT = pb.tile([128, 2], F32)
                wbr = nc.sync.dma_start(wT[:], wb_d.ap().rearrange("(t p) -> p t", p=128))
                add_dep_helper(wbr.ins, wbw.ins, reason="w bounce order")

                # pooledT = sum_t' w[t'] v[t', :]   -> [1, 8]
                psp = psb.tile([1, 8], F32, tag="psb_p")
                for tcol in range(2):
                    nc.tensor.matmul(psp[:], lhsT=wT[:, tcol:tcol + 1],
                                     rhs=v_sb[:, tcol], start=(tcol == 0),
                                     stop=(tcol == 1))

                # final softmax over 8 logits (pooled = psp / 200)
                mm = pb.tile([1, 1], F32)
                nc.vector.reduce_max(mm[:], psp[:], axis=mybir.AxisListType.X)
                mneg8 = pb.tile([1, 1], F32)
                nc.vector.tensor_scalar_mul(mneg8[:], mm[:], -1.0 / 200.0)
                e8 = pb.tile([1, 8], F32)
                s8 = pb.tile([1, 1], F32)
                nc.scalar.activation(e8[:], psp[:], AF.Exp, bias=mneg8[:],
                                     scale=1.0 / 200.0, accum_out=s8[:])
                r8 = pb.tile([1, 1], F32)
                nc.vector.reciprocal(r8[:], s8[:])
                outv = pb.tile([1, 8], F32)
                nc.vector.tensor_scalar(outv[:], e8[:], r8[:], None, op0=ALU.mult)
                nc.sync.dma_start(out_d.ap(), outv[:])

    nc.compile()
    return nc


def kernel(**inputs):
    key = 'nc'
    if key not in _CACHE:
        _CACHE[key] = build_bass()
    nc = _CACHE[key]

    w = _fold_weights(inputs)
    x = np.ascontiguousarray(np.asarray(inputs['x'], dtype=np.float32))
    in_maps = []
    for b in range(B):
        m = {'x': x[b]}
        for name, shape, dt in WEIGHT_SPECS:
            m[name] = w[name]
        in_maps.append(m)

    res = bass_utils.run_bass_kernel_spmd(nc, in_maps, core_ids=list(range(B)))
    LAST['res'] = res
    out = np.stack([res.results[b]['out'][0] for b in range(B)], axis=0)
    return out.astype(np.float32)


if __name__ == '__main__':
    nc = build_bass()
    print("build ok:", len(nc.m.functions[0].blocks), "blocks")



# revision 7
# speedup vs baseline: 1.1606x; 1.1606x over previous
"""Trainium2 Bass kernel for nn_AMIML_5102421148136 (topk_masking).

Data-parallel over batch B=8 across 8 NeuronCores. Each core runs, for its
sample:
  Phase A (bulk): MLP 256->128->64->32 (BN folded, relu) -> score projection
    (conv4+conv5 folded) over all 100k positions, bf16 compute with fp32
    accumulation. Scores written to a DRAM scratch.
  Phase B: gpsimd global top-256 of s and of -s; gather the 512 candidate
    x-rows; recompute the MLP on candidates in fp32(r); exact top/bottom-100
    selection among candidates (max8/match_replace rounds); tiny 200-token
    attention; final softmax -> out row [1, 8].
Host stacks the 8 per-core rows into the [8, 8] output.
"""

import sys
import math

if '/opt/trn_rl_repo' not in sys.path:
    sys.path.insert(0, '/opt/trn_rl_repo')

import numpy as np
import ml_dtypes

import concourse.bass as bass
import concourse.mybir as mybir
from concourse.tile import TileContext
from concourse.tile_rust import add_dep_helper
from concourse import bass_utils, bacc

F32 = mybir.dt.float32
F32R = mybir.dt.float32r
BF16 = mybir.dt.bfloat16
F8 = mybir.dt.float8e4
U32 = mybir.dt.uint32
AF = mybir.ActivationFunctionType
ALU = mybir.AluOpType

# ---- problem constants (hardcoded; kernel.py must be self-contained) ----
B = 8
N = 100000
NPAD = 100352          # 16 * 6272 = 112 * 896, multiple of 128
C = 256
TILE = 896             # positions per phase-A tile (7 * 128)
NT = NPAD // TILE      # 112
GRP = 7                # tiles per topk partition chunk (7*896 = 6272)
CHUNK = NPAD // 16     # 6272 positions per topk partition
D = 8
R = 100
ALPHA = 0.1
EPS = 1e-5
NEG = -1.0e30
TOK = 256              # padded token count for attention (200 real)
HALF = NPAD // 2       # 50176, topk vocab per token (ISA limit: u16)
CHUNK2 = HALF // 16    # 3136
NCAND = 512            # 4 tokens x top-128 kept

_CACHE = {}
LAST = {}


def _fold_weights(inputs):
    """Host-side exact algebra: BN fold, conv4+conv5 fold, bias augmentation."""
    w = {}

    def fold(li):
        cw = inputs[f'conv{li}_w'].astype(np.float64)
        cb = inputs[f'conv{li}_b'].astype(np.float64)
        g = inputs[f'bn{li}_g'].astype(np.float64)
        bb = inputs[f'bn{li}_b'].astype(np.float64)
        m = inputs[f'bn{li}_m'].astype(np.float64)
        v = inputs[f'bn{li}_v'].astype(np.float64)
        sc = g / np.sqrt(v + EPS)
        Wf = (cw * sc[:, None]).T          # [cin, cout]
        bf = (cb - m) * sc + bb
        return Wf.astype(np.float32), bf.astype(np.float32)

    W1, b1 = fold(1)
    W2, b2 = fold(2)
    W3, b3 = fold(3)
    W4 = inputs['conv4_w'].T.astype(np.float32)       # [32, 8]
    b4 = inputs['conv4_b'].astype(np.float32)         # [8]
    w5 = inputs['conv5_w'][0].astype(np.float32)      # [8]
    b5 = float(inputs['conv5_b'][0])
    Ws = (W4.astype(np.float64) @ w5.astype(np.float64)).astype(np.float32)  # [32]
    bs = float(b4.astype(np.float64) @ w5.astype(np.float64) + b5)

    # phase A weights, SBUF layouts. L1 runs in fp8 on pair-packed
    # transposed x: partition p holds channels (2p, 2p+1); parity matmuls
    # contract over one channel parity each.
    f8 = ml_dtypes.float8_e4m3
    w['w1e'] = W1[0::2, :].astype(f8)                  # [128, 128] fp8
    w['w1o'] = W1[1::2, :].astype(f8)                  # [128, 128] fp8
    w['w2'] = W2.astype(ml_dtypes.bfloat16)            # [128, 64]
    w['w3'] = W3.astype(ml_dtypes.bfloat16)            # [64, 32]
    w['wsc'] = Ws.reshape(32, 1).astype(ml_dtypes.bfloat16)  # [32, 1]
    w['b1'] = b1.reshape(128, 1)
    w['b2'] = b2.reshape(64, 1)
    w['b3'] = b3.reshape(32, 1)

    # refine (fp32) weights
    w['rw1'] = W1.reshape(2, 128, 128).transpose(1, 0, 2).copy()  # [128,2,128] f32
    w['rw2'] = W2.copy()                               # [128, 64]
    w['rw3'] = W3.copy()                               # [64, 32]
    rw45 = np.concatenate([W4, Ws.reshape(32, 1)], axis=1)  # [32, 9]
    w['rw45'] = rw45.astype(np.float32)
    rb45 = np.concatenate([b4, np.array([bs], np.float32)]).reshape(9, 1)
    w['rb45'] = rb45.astype(np.float32)

    # attention weights. scores scaled by 1/sqrt(D) folded into q.
    sq = 1.0 / math.sqrt(D)
    qw = inputs['q_w'].astype(np.float32) * sq         # [8, 8] (out, in)
    qb = inputs['q_b'].astype(np.float32) * sq
    kw = inputs['k_w'].astype(np.float32)
    kb = inputs['k_b'].astype(np.float32)
    vw = inputs['v_w'].astype(np.float32)
    vb = inputs['v_b'].astype(np.float32)
    # augmented lhsT [9, 8]: rows 0..7 = W.T (d, e), row 8 = bias
    w['qw'] = np.concatenate([qw.T, qb.reshape(1, 8)], axis=0).astype(np.float32)
    w['kw'] = np.concatenate([kw.T, kb.reshape(1, 8)], axis=0).astype(np.float32)
    w['vw'] = np.concatenate([vw.T, vb.reshape(1, 8)], axis=0).astype(np.float32)
    w['w6'] = (ALPHA * inputs['conv6_w'].astype(np.float32)).reshape(1, 8)
    w['b6'] = (ALPHA * inputs['conv6_b'].astype(np.float32)).reshape(1, 8)

    # constants
    ident = np.eye(128, dtype=np.float32)
    w['ident'] = ident
    w['ident16'] = ident.astype(ml_dtypes.bfloat16)
    onesmask = np.zeros((128, 2), np.float32)
    onesmask[:, 0] = 1.0
    onesmask[:72, 1] = 1.0
    w['onesmask'] = onesmask
    w['c256'] = np.array([[0.0], [256.0]], np.float32)  # per-partition col offset
    w['negrow'] = np.full((1, 352), NEG, np.float32)
    offc = np.zeros((128, 4), np.float32)
    for tcol in range(4):
        offc[:, tcol] = float(tcol % 2) * (100352 // 2)
    w['offc'] = offc
    w['sgn'] = np.array([[1.0], [-1.0]], np.float32)   # val sign per side
    return w


WEIGHT_SPECS = [
    ('w1e', [128, 128], F8), ('w1o', [128, 128], F8),
    ('w2', [128, 64], BF16), ('w3', [64, 32], BF16),
    ('wsc', [32, 1], BF16),
    ('b1', [128, 1], F32), ('b2', [64, 1], F32), ('b3', [32, 1], F32),
    ('rw1', [128, 2, 128], F32), ('rw2', [128, 64], F32), ('rw3', [64, 32], F32),
    ('rw45', [32, 9], F32), ('rb45', [9, 1], F32),
    ('qw', [9, 8], F32), ('kw', [9, 8], F32), ('vw', [9, 8], F32),
    ('w6', [1, 8], F32), ('b6', [1, 8], F32),
    ('ident', [128, 128], F32), ('ident16', [128, 128], BF16),
    ('onesmask', [128, 2], F32),
    ('c256', [2, 1], F32), ('sgn', [2, 1], F32), ('offc', [128, 4], F32),
    ('negrow', [1, 352], F32),
]


def build_bass():
    nc = bacc.Bacc("TRN2", target_bir_lowering=False, debug=False)

    x_d = nc.dram_tensor("x", [N, C], F32, kind="ExternalInput")
    wd = {}
    for name, shape, dt in WEIGHT_SPECS:
        wd[name] = nc.dram_tensor(name, shape, dt, kind="ExternalInput")
    out_d = nc.dram_tensor("out", [1, D], F32, kind="ExternalOutput")

    s_d = nc.dram_tensor("s_scratch", [NPAD], F32, kind="Internal")
    cand_d = nc.dram_tensor("cand_scratch", [NCAND, 16], F32, kind="Internal")
    idx_d = nc.dram_tensor("idx_scratch", [1024], U32, kind="Internal")
    sc_d = nc.dram_tensor("sc_scratch", [NCAND], F32, kind="Internal")
    vi_d = nc.dram_tensor("vi_scratch", [208], U32, kind="Internal")
    vv_d = nc.dram_tensor("vv_scratch", [208], F32, kind="Internal")
    wb_d = nc.dram_tensor("w_scratch", [256], F32, kind="Internal")

    with TileContext(nc) as tc:
        with tc.tile_pool(name="consts", bufs=1) as cpool:
            ws = {}
            for name, shape, dt in WEIGHT_SPECS:
                t = cpool.tile(shape, dt, tag=name)
                sl = t[0:shape[0]]
                nc.sync.dma_start(sl, wd[name].ap())
                ws[name] = t[0:shape[0]]

            # ---------------- Phase A ----------------
            # raw SBUF tensors (gpsimd.topk requires SBTensorHandle)
            s2_t = nc.alloc_sbuf_tensor("s2_topk", [64, CHUNK2], F32)
            s2_sb = s2_t.ap()
            tk_t = nc.alloc_sbuf_tensor("tk_out", [64, 32], U32)

            with (
                tc.tile_pool(name="xin", bufs=4) as xpool,
                tc.tile_pool(name="work", bufs=3) as wpool,
                tc.tile_pool(name="sstage", bufs=2) as spool,
                tc.tile_pool(name="ps_xt", bufs=2, space="PSUM") as ps_xt,
                tc.tile_pool(name="ps_big", bufs=1, space="PSUM") as ps_big,
                tc.tile_pool(name="ps_s", bufs=1, space="PSUM") as ps_sp,
            ):
                s_stage = None
                ps_s = ps_sp.tile([128, 1024], F32, tag="pss")
                saved = {}
                s_writes = []

                def load_x(t_i):
                    p0 = t_i * TILE
                    x_sb = xpool.tile([128, 7, C], F8, tag="x")
                    if p0 + TILE <= N:
                        nc.gpsimd.dma_start(
                            x_sb[:],
                            x_d.ap()[p0:p0 + TILE, :].rearrange("(g p) c -> p g c", p=128),
                        )
                    else:
                        nval = N - p0          # 544 = 4*128 + 32
                        gfull = nval // 128    # 4
                        rem = nval - gfull * 128
                        nc.vector.memset(x_sb[:], 0.0)
                        nc.gpsimd.dma_start(
                            x_sb[:, :gfull],
                            x_d.ap()[p0:p0 + gfull * 128, :].rearrange(
                                "(g p) c -> p g c", p=128),
                        )
                        if rem:
                            nc.gpsimd.dma_start(
                                x_sb[:rem, gfull],
                                x_d.ap()[p0 + gfull * 128:p0 + nval, :],
                            )
                    return x_sb

                for t_i in range(NT + 1):
                    cur = t_i < NT
                    prv = t_i >= 1

                    if prv:
                        # L2 (t_i-1): [128 -> 64]
                        h1p = saved['h1']
                        ph2 = ps_big.tile([64, TILE], F32, tag="ps23")
                        for nn_ in range(2):
                            sl = slice(nn_ * 448, (nn_ + 1) * 448)
                            nc.tensor.matmul(ph2[:, sl], lhsT=ws['w2'],
                                             rhs=h1p[:, sl], start=True, stop=True)
                        h2 = wpool.tile([64, TILE], BF16, tag="h2")
                        nc.scalar.activation(h2[:], ph2[:], AF.Relu, bias=ws['b2'])

                    if cur:
                        x_sb = load_x(t_i)
                        # transpose x tile to channel-pair-major via PE: each
                        # 16-bit element packs fp8 channels (2p, 2p+1); one
                        # 128x128 transpose covers all 256 channels of a
                        # 128-position group.
                        xT = wpool.tile([128, TILE], BF16, tag="xT")
                        pst = ps_xt.tile([128, TILE], BF16, tag="psxT")
                        for g in range(7):
                            nc.tensor.transpose(
                                pst[:, g * 128:(g + 1) * 128],
                                x_sb[:, g].bitcast(BF16),
                                ws['ident16'],
                            )
                        nc.vector.tensor_copy(xT[:], pst[:])

                    if prv:
                        # L3 (t_i-1): [64 -> 32]
                        ph3 = ps_big.tile([32, TILE], F32, tag="ps23")
                        for nn_ in range(2):
                            sl = slice(nn_ * 448, (nn_ + 1) * 448)
                            nc.tensor.matmul(ph3[:, sl], lhsT=ws['w3'],
                                             rhs=h2[:, sl], start=True, stop=True)
                        h3 = wpool.tile([32, TILE], BF16, tag="h3")
                        nc.vector.tensor_scalar(h3[:], ph3[:], ws['b3'], 0.0,
                                                op0=ALU.add, op1=ALU.max)

                    if cur:
                        # L1 (t_i): [256 -> 128], fp8 parity matmuls over the
                        # pair-packed xT (stride-2 fp8 rhs)
                        ph1 = ps_big.tile([128, TILE], F32, tag="ps1")
                        xT8 = xT[:].bitcast(F8).rearrange(
                            "p (n two) -> p n two", two=2)
                        for nn_ in range(2):
                            sl = slice(nn_ * 448, (nn_ + 1) * 448)
                            for par, wname in ((0, 'w1e'), (1, 'w1o')):
                                nc.tensor.matmul(
                                    ph1[:, sl], lhsT=ws[wname],
                                    rhs=xT8[:, sl, par],
                                    start=(par == 0), stop=(par == 1),
                                )
                        h1 = wpool.tile([128, TILE], BF16, tag="h1")
                        nc.scalar.activation(h1[:], ph1[:], AF.Relu, bias=ws['b1'])
                        saved['h1'] = h1

                    if prv:
                        tp = t_i - 1
                        # score projection [32 -> 1]: chunk c of the current
                        # 4-tile group at psum partition 32*(c%4), bank c//4
                        if tp == 0:
                            nc.vector.memset(ps_s[:], 0.0)
                        for nn_ in range(2):
                            sl = slice(nn_ * 448, (nn_ + 1) * 448)
                            c = 2 * (tp % 4) + nn_
                            pb_ = 32 * (c % 4)
                            bk = c // 4
                            nc.tensor.matmul(
                                ps_s[pb_:pb_ + 1, bk * 512:bk * 512 + 448],
                                lhsT=ws['wsc'],
                                rhs=h3[:, sl], start=True, stop=True,
                                tile_position=(0, pb_),
                            )
                        if tp % 4 == 3:
                            gi = tp // 4
                            s_stage = spool.tile([128, 2, 448], F32, tag="sstage")
                            for bk_ in range(2):
                                nc.vector.tensor_copy(
                                    s_stage[:, bk_],
                                    ps_s[:, bk_ * 512:bk_ * 512 + 448])
                            sd_ap = s_d.ap()
                            for p4 in range(4):
                                off = gi * 3584 + p4 * 448
                                w_i = nc.sync.dma_start(
                                    bass.AP(sd_ap.tensor, off, [[1792, 2], [1, 448]]),
                                    s_stage[32 * p4:32 * p4 + 1, :, :],
                                )
                                s_writes.append(w_i.ins)

            # ---------------- Phase B ----------------
            with (
                tc.tile_pool(name="pb", bufs=1) as pb,
                tc.tile_pool(name="ps_b", bufs=1, space="PSUM") as psb,
            ):
                # topk input: tokens 0,1 = s halves; tokens 2,3 = -s halves
                ha = s_d.ap().rearrange("(p f) -> p f", p=32)
                ld0 = nc.sync.dma_start(s2_sb[0:32], ha)
                ld1 = nc.sync.dma_start(s2_sb[32:64], ha)
                for w_ in s_writes:
                    add_dep_helper(ld0.ins, w_, reason="s2 load after score writes")
                    add_dep_helper(ld1.ins, w_, reason="s2 load after score writes")
                neg_i = nc.vector.tensor_scalar_mul(s2_sb[32:64], s2_sb[32:64], -1.0)
                add_dep_helper(neg_i.ins, ld0.ins, reason="neg after load")
                add_dep_helper(neg_i.ins, ld1.ins, reason="neg after load")
                pad0 = N - 31 * CHUNK2   # 2784: valid prefix in partition 31/63
                # DVE can't address partition bases 31/63; patch pads via DMA
                pt0 = nc.sync.dma_start(s2_sb[31:32, pad0:], ws['negrow'])
                pt1 = nc.sync.dma_start(s2_sb[63:64, pad0:], ws['negrow'])
                add_dep_helper(pt0.ins, ld0.ins, reason="patch after load")
                add_dep_helper(pt1.ins, neg_i.ins, reason="patch after neg")

                tk = tk_t.ap()
                tk_i = nc.gpsimd.topk(tk, s2_sb, tokens=4, vocab_size=HALF, k=256)
                for d_ in (ld0, ld1, neg_i, pt0, pt1):
                    add_dep_helper(tk_i.ins, d_.ins, reason="topk after s2 ready")

                # rearrange candidate indices via DRAM bounce; keep each
                # token's top-128 (ascending sort: slots 128..255)
                wi_ = nc.sync.dma_start(
                    idx_d.ap().rearrange("(p f) -> p f", p=64), tk[:, 16:32])
                add_dep_helper(wi_.ins, tk_i.ins, reason="idx write after topk")
                idxg = pb.tile([128, 4], U32)
                ida = idx_d.ap()
                ri_ = nc.sync.dma_start(
                    idxg[:], bass.AP(ida.tensor, 128, [[1, 128], [256, 4]]))
                add_dep_helper(ri_.ins, wi_.ins, reason="idx bounce order")
                # add per-half position offset (via f32; values < 2^24 exact)
                idxf = pb.tile([128, 4], F32)
                nc.vector.tensor_copy(idxf[:], idxg[:])
                nc.vector.tensor_add(idxf[:], idxf[:], ws['offc'])
                nc.vector.tensor_copy(idxg[:], idxf[:])

                # gather candidate x rows (512 rows of 256 floats)
                xg = pb.tile([128, 4, C], F32)
                xg_gathers = []
                for tcol in range(4):
                    g0 = nc.gpsimd.indirect_dma_start(
                        out=xg[:, tcol], out_offset=None,
                        in_=x_d.ap(),
                        in_offset=bass.IndirectOffsetOnAxis(ap=idxg[:, tcol:tcol + 1], axis=0),
                    )
                    xg_gathers.append(g0)

                # transpose candidates to channel-major fp32
                xgT = pb.tile([128, 2, NCAND], F32)
                for ch in range(1):
                    for cc in range(2):
                        pst = psb.tile([128, 512], F32, tag="psb512")
                        for tq in range(4):
                            tcol = ch * 4 + tq
                            tr_i = nc.tensor.transpose(
                                pst[:, tq * 128:(tq + 1) * 128],
                                xg[:, tcol, cc * 128:(cc + 1) * 128],
                                ws['ident'],
                            )
                            add_dep_helper(tr_i.ins, xg_gathers[tcol].ins,
                                           reason="transpose after gather")
                        nc.vector.tensor_copy(
                            xgT[:, cc, ch * 512:(ch + 1) * 512], pst[:])

                # refine MLP in fp32, 512-wide chunks
                r45 = pb.tile([128, NCAND], F32)  # rows 0..7 x4T, row 8 exact s
                nc.vector.memset(r45[:], 0.0)
                for ch in range(1):
                    sl = slice(ch * 512, (ch + 1) * 512)
                    ps1 = psb.tile([128, 512], F32, tag="psb512")
                    for cc in range(2):
                        nc.tensor.matmul(
                            ps1[:], lhsT=ws['rw1'][:, cc],
                            rhs=xgT[:, cc, sl],
                            start=(cc == 0), stop=(cc == 1))
                    rh1 = pb.tile([128, 512], F32, tag="rh1")
                    nc.scalar.activation(rh1[:], ps1[:], AF.Relu, bias=ws['b1'])

                    ps2 = psb.tile([64, 512], F32, tag="psb512")
                    nc.tensor.matmul(ps2[:], lhsT=ws['rw2'],
                                     rhs=rh1[:], start=True, stop=True)
                    rh2 = pb.tile([64, 512], F32, tag="rh2")
                    nc.scalar.activation(rh2[:], ps2[:], AF.Relu, bias=ws['b2'])

                    ps3 = psb.tile([32, 512], F32, tag="psb512")
                    nc.tensor.matmul(ps3[:], lhsT=ws['rw3'],
                                     rhs=rh2[:], start=True, stop=True)
                    rh3 = pb.tile([32, 512], F32, tag="rh3")
                    nc.scalar.activation(rh3[:], ps3[:], AF.Relu, bias=ws['b3'])

                    ps4 = psb.tile([9, 512], F32, tag="psb512")
                    nc.tensor.matmul(ps4[:], lhsT=ws['rw45'],
                                     rhs=rh3[:], start=True, stop=True)
                    nc.vector.tensor_scalar(r45[0:9, sl], ps4[:], ws['rb45'],
                                            None, op0=ALU.add)

                # store candidates row-major for the column gather:
                # [512 cand, 16] (cols 0..8 = x4|s, rest pad)
                r45T = pb.tile([128, 4, 16], F32)
                nc.vector.memset(r45T[:], 0.0)
                for ch in range(1):
                    pst45 = psb.tile([128, 512], F32, tag="psb512")
                    for tq in range(4):
                        tcol = ch * 4 + tq
                        nc.tensor.transpose(
                            pst45[:, tq * 128:(tq + 1) * 128],
                            r45[:, tcol * 128:(tcol + 1) * 128],
                            ws['ident'],
                        )
                    for tq in range(4):
                        tcol = ch * 4 + tq
                        nc.vector.tensor_copy(
                            r45T[:, tcol, 0:9], pst45[:, tq * 128:tq * 128 + 9])
                candw = nc.sync.dma_start(
                    cand_d.ap().rearrange("(t p) f -> p t f", p=128), r45T[:])

                # exact select: top-100 of s (row 0) and of -s (row 1) among
                # 256 candidates each
                s2c = pb.tile([2, 256], F32)
                scw = nc.sync.dma_start(sc_d.ap().rearrange("(o f) -> o f", o=1), r45[8:9, :])
                scr = nc.sync.dma_start(s2c[:], sc_d.ap().rearrange("(t c) -> t c", t=2))
                add_dep_helper(scr.ins, scw.ins, reason="sc bounce order")
                nc.vector.tensor_scalar(s2c[:], s2c[:], ws['sgn'], None, op0=ALU.mult)

                work = pb.tile([2, 256], F32)
                nc.vector.tensor_copy(work[:], s2c[:])
                vals = pb.tile([2, 104], F32)
                cidx = pb.tile([2, 104], U32)
                for r_ in range(13):
                    sl = slice(r_ * 8, (r_ + 1) * 8)
                    nc.vector.max(out=vals[:, sl], in_=work[:])
                    nc.vector.max_index(out=cidx[:, sl], in_max=vals[:, sl],
                                        in_values=s2c[:])
                    nc.vector.match_replace(out=work[:], in_to_replace=vals[:, sl],
                                            in_values=work[:], imm_value=NEG)

                # global candidate row = side*256 + cidx ; vals row1 = -vals
                cidx_f = pb.tile([2, 104], F32)
                nc.vector.tensor_copy(cidx_f[:], cidx[:])
                nc.vector.tensor_scalar(cidx_f[:], cidx_f[:], ws['c256'], None, op0=ALU.add)
                ccol = pb.tile([2, 104], U32)
                nc.vector.tensor_copy(ccol[:], cidx_f[:])
                nc.vector.tensor_scalar(vals[:], vals[:], ws['sgn'], None, op0=ALU.mult)

                # build token-order index [128, 2] and values [128, 2]
                ccol_g = pb.tile([128, 2], U32)
                vals_g = pb.tile([128, 2], F32)
                for (dst, src_t, bd) in ((ccol_g, ccol, vi_d), (vals_g, vals, vv_d)):
                    bw_ = nc.sync.dma_start(bd.ap().rearrange("(p f) -> p f", p=2), src_t[:])
                    bda = bd.ap()
                    for (osl, isl) in (((slice(0, 100), 0), (0, 100)),
                                       ((slice(100, 128), 0), (104, 132)),
                                       ((slice(0, 72), 1), (132, 204)),
                                       ((slice(72, 128), 1), (148, 204))):
                        r_i = nc.sync.dma_start(
                            dst[osl[0], osl[1]:osl[1] + 1],
                            bda[isl[0]:isl[1]].rearrange("f -> f ()"))
                        add_dep_helper(r_i.ins, bw_.ins, reason="vi/vv bounce order")

                # gather selected candidate rows: [128, 2, 16]
                x4a = pb.tile([128, 2, 16], F32)
                x4a_gathers = []
                for tcol in range(2):
                    g_i = nc.gpsimd.indirect_dma_start(
                        out=x4a[:, tcol], out_offset=None,
                        in_=cand_d.ap(),
                        in_offset=bass.IndirectOffsetOnAxis(ap=ccol_g[:, tcol:tcol + 1], axis=0),
                    )
                    add_dep_helper(g_i.ins, candw.ins, reason="gather after cand write")
                    x4a_gathers.append(g_i)

                # h2 tokens (token-major): x4 + vals*w6 + b6 ; col 8 = 1.0
                w6b = pb.tile([128, 8], F32)
                nc.gpsimd.partition_broadcast(w6b[:], ws['w6'])
                b6b = pb.tile([128, 8], F32)
                nc.gpsimd.partition_broadcast(b6b[:], ws['b6'])

                h2a = pb.tile([128, 2, 9], F32)
                for tcol in range(2):
                    nc.vector.tensor_scalar(h2a[:, tcol, 0:8], w6b[:],
                                            vals_g[:, tcol:tcol + 1], None, op0=ALU.mult)
                    a_i = nc.vector.tensor_add(h2a[:, tcol, 0:8], h2a[:, tcol, 0:8],
                                               x4a[:, tcol, 0:8])
                    add_dep_helper(a_i.ins, x4a_gathers[tcol].ins,
                                   reason="h2 after x4a gather")
                    nc.vector.tensor_add(h2a[:, tcol, 0:8], h2a[:, tcol, 0:8], b6b[:])
                nc.vector.memset(h2a[:, :, 8:9], 1.0)

                # transpose h2 to channel-major [9, 256]
                h2T = pb.tile([9, TOK], F32)
                psh = psb.tile([16, 256], F32, tag="psb_h2t")
                for tcol in range(2):
                    nc.tensor.transpose(
                        psh[0:9, tcol * 128:(tcol + 1) * 128],
                        h2a[:, tcol], ws['ident'])
                nc.vector.tensor_copy(h2T[:], psh[0:9, :])

                # q, k (channel-major) and v (token-major)
                psq = psb.tile([8, TOK], F32, tag="psb_q")
                nc.tensor.matmul(psq[:], lhsT=ws['qw'], rhs=h2T[:], start=True, stop=True)
                qT = pb.tile([8, TOK], F32)
                nc.vector.tensor_copy(qT[:], psq[:])

                psk = psb.tile([8, TOK], F32, tag="psb_q")
                nc.tensor.matmul(psk[:], lhsT=ws['kw'], rhs=h2T[:], start=True, stop=True)
                kT = pb.tile([8, TOK], F32)
                nc.vector.tensor_copy(kT[:], psk[:])

                v_sb = pb.tile([128, 2, 8], F32)
                for tcol in range(2):
                    psv = psb.tile([128, 8], F32, tag="psb_v")
                    nc.tensor.matmul(psv[:], lhsT=h2T[:, tcol * 128:(tcol + 1) * 128],
                                     rhs=ws['vw'], start=True, stop=True)
                    nc.vector.tensor_copy(v_sb[:, tcol], psv[:])

                # scores + masked softmax + column sums (weights w)
                psw = psb.tile([1, TOK], F32, tag="psb_w")
                for tcol in range(2):
                    psS = psb.tile([128, TOK], F32, tag="psb_S")
                    nc.tensor.matmul(psS[:], lhsT=qT[:, tcol * 128:(tcol + 1) * 128],
                                     rhs=kT[:], start=True, stop=True)
                    nc.vector.memset(psS[:, 200:], NEG)
                    mrow = pb.tile([128, 1], F32, tag="mrow")
                    nc.vector.reduce_max(mrow[:], psS[:], axis=mybir.AxisListType.X)
                    mneg = pb.tile([128, 1], F32, tag="mneg")
                    nc.vector.tensor_scalar_mul(mneg[:], mrow[:], -1.0)
                    pexp = pb.tile([128, TOK], F32, tag="pexp")
                    sume = pb.tile([128, 1], F32, tag="sume")
                    nc.scalar.activation(pexp[:], psS[:], AF.Exp, bias=mneg[:],
                                         accum_out=sume[:])
                    rsum = pb.tile([128, 1], F32, tag="rsum")
                    nc.vector.reciprocal(rsum[:], sume[:])
                    nc.vector.tensor_mul(rsum[:], rsum[:],
                                         ws['onesmask'][:, tcol:tcol + 1])
                    nc.tensor.matmul(psw[:], lhsT=rsum[:], rhs=pexp[:],
                                     start=(tcol == 0), stop=(tcol == 1))

                w_sb = pb.tile([1, TOK], F32)
                nc.vector.tensor_copy(w_sb[:], psw[:])
                wbw = nc.sync.dma_start(wb_d.ap().rearrange("(o f) -> o f", o=1), w_sb[:])
                wT = pb.tile([128, 2], F32)
                wbr = nc.sync.dma_start(wT[:], wb_d.ap().rearrange("(t p) -> p t", p=128))
                add_dep_helper(wbr.ins, wbw.ins, reason="w bounce order")

                # pooledT = sum_t' w[t'] v[t', :]   -> [1, 8]
                psp = psb.tile([1, 8], F32, tag="psb_p")
                for tcol in range(2):
                    nc.tensor.matmul(psp[:], lhsT=wT[:, tcol:tcol + 1],
                                     rhs=v_sb[:, tcol], start=(tcol == 0),
                                     stop=(tcol == 1))

                # final softmax over 8 logits (pooled = psp / 200)
                mm = pb.tile([1, 1], F32)
                nc.vector.reduce_max(mm[:], psp[:], axis=mybir.AxisListType.X)
                mneg8 = pb.tile([1, 1], F32)
                nc.vector.tensor_scalar_mul(mneg8[:], mm[:], -1.0 / 200.0)
                e8 = pb.tile([1, 8], F32)
                s8 = pb.tile([1, 1], F32)
                nc.scalar.activation(e8[:], psp[:], AF.Exp, bias=mneg8[:],
                                     scale=1.0 / 200.0, accum_out=s8[:])
                r8 = pb.tile([1, 1], F32)
                nc.vector.reciprocal(r8[:], s8[:])
                outv = pb.tile([1, 8], F32)
                nc.vector.tensor_scalar(outv[:], e8[:], r8[:], None, op0=ALU.mult)
                nc.sync.dma_start(out_d.ap(), outv[:])

    nc.compile()
    return nc


def kernel(**inputs):
    key = 'nc'
    if key not in _CACHE:
        _CACHE[key] = build_bass()
    nc = _CACHE[key]

    w = _fold_weights(inputs)
    x = np.ascontiguousarray(np.asarray(inputs['x'], dtype=np.float32))
    in_maps = []
    for b in range(B):
        m = {'x': x[b]}
        for name, shape, dt in WEIGHT_SPECS:
            m[name] = w[name]
        in_maps.append(m)

    res = bass_utils.run_bass_kernel_spmd(nc, in_maps, core_ids=list(range(B)))
    LAST['res'] = res
    out = np.stack([res.results[b]['out'][0] for b in range(B)], axis=0)
    return out.astype(np.float32)


if __name__ == '__main__':
    nc = build_bass()
    print("build ok:", len(nc.m.functions[0].blocks), "blocks")

